# revision 44
# baseline (speedup 1.0000x reference)
"""BERT-CRF Viterbi decode kernel for Trainium2 (Bass/Tile), 8-core data parallel.

v3: transitions folded into the PE matmul; batched group-level max-plus
algebra with unit-stride compose ops throughout.

Full inputs in, full outputs out. Batch B=64 sharded across 8 cores (8 seqs
each). Per core, 128 partition rows = (b=8 seqs) x (c=16 chunks of L=32 steps);
each chunk splits into NG=8 groups of UG=4 steps.

  Host pre-transposes sentences to fp16 [p=h%128, (g, ch, uu, row)] and packs
  W as fp16 hi+lo (32 cols per h-chunk), so one PE pass per group produces
  eT[(j,k)hi|lo, (uu,row)] with trans[k,j]+b[j] riding a tiny hi/lo seed
  matmul against a DMA-loaded ones-row. PE transposes ([32,128] blocks)
  deliver per-row layout; one DVE add (hi+lo) finishes the fp32-grade step
  matrices TS[u][j][k] = T_u^T.

  All max-plus composes C = A o B are emitted as t[a,b,m] = A[a,m] + B^T[b,m]
  with unit innermost stride on BOTH operands and reduce_max over m -- the
  fast DVE pattern. Chains keep the recurrent operand on the A side; the
  static operand is pre-transposed (TS is already T^T; TN/PTs/STc2 come from
  cheap Scalar strided copies that hide under the DMA stream).

  Per group (hidden under the stream): P chain (3), S^T chain (2), running
  cross-group GpreN (1) = 6 composes. Tail: GsufTc (7 composes), boundary
  scans p2f/p2b over chunks, batched asb/bsb -> alpha/beta backfill ->
  tags = first-argmax_j(alpha_u[j] + beta_u[j]).
"""
import sys
for p in ("/opt/trn_rl_repo", "/root/.axon_site/_ro/trn_rl_repo"):
    if p not in sys.path:
        sys.path.append(p)

import numpy as np
import concourse.bass as bass
import concourse.tile as tile
from concourse import mybir
from concourse.bass_utils import run_bass_kernel_spmd

F32 = mybir.dt.float32
FP16 = mybir.dt.float16
I32 = mybir.dt.int32
AX = mybir.AxisListType
OP = mybir.AluOpType

B, T, H, K = 64, 512, 768, 4
NCORES = 8
BC = B // NCORES          # 8 sequences per core
C, L = 16, 32             # chunks per sequence, steps per chunk
ROWS = BC * C             # 128 partition rows
HCH = H // 128            # 6 h-chunks
UG = 4                    # steps per group
NG = L // UG              # 8 groups
GW = HCH * UG * 128       # 3072 fp16 cols per partition per group
WCOLS = HCH * 32          # W' lhsT cols: 6 chunks x (16 hi | 16 lo)

NEG = -1.0e30

_NC_CACHE = {}


def build_nc():
    nc = bass.Bass()
    sentd = nc.declare_dram_parameter("sentT", [128, NG * GW], FP16, isOutput=False)
    wtd = nc.declare_dram_parameter("wt", [128, WCOLS], FP16, isOutput=False)
    # rowconsts[128, 128]: wfirst | iw4 | mpid | end | rcfix | id32 | tbrow
    rcd = nc.declare_dram_parameter("rowconsts", [128, 128], F32, isOutput=False)
    tagsd = nc.declare_dram_parameter("tags", [BC, T], I32, isOutput=True)

    with tile.TileContext(nc) as tc:
        with tc.tile_pool(name="singles", bufs=1) as singles, \
             tc.tile_pool(name="gpool", bufs=NG) as gpool, \
             tc.tile_pool(name="et_pool", bufs=3) as et_pool, \
             tc.tile_pool(name="tmp_pool", bufs=2) as tmp_pool, \
             tc.tile_pool(name="gtmp_pool", bufs=2) as gtmp_pool, \
             tc.tile_pool(name="ps_eT", bufs=3, space="PSUM") as ps_eT, \
             tc.tile_pool(name="ps_fx", bufs=2, space="PSUM") as ps_fx:

            # ---------- constants (Act ring; Sync ring leads with group 0) ----
            wt = singles.tile([128, WCOLS], FP16)
            nc.scalar.dma_start(wt, wtd[:])
            rc = singles.tile([128, 128], F32)
            nc.scalar.dma_start(rc, rcd[:])
            wfirst = rc[:, 0:4]
            iw4 = rc[:, 4:8]
            mpid = rc[:, 8:24]          # max-plus identity (0 diag, NEG off)
            end_sb = rc[:, 24:28]
            rcfix = rc[:, 28:44]        # rows c==0: start[j]-trans[k,j]; else 0
            id32 = rc[0:32, 44:76]
            tbrow = rc[:, 92:108]       # tbrow[j*4+k] = trans[k,j] + b[j]
            tiebias = rc[:, 108:112]    # [-j * 2^-17 for j in 0..3]

            # ---------- prefetch all sentence groups ----------
            # Groups 0 and 7 land in 3 separate ch-pair tiles each so their
            # matmuls start/finish as soon as each h-chunk arrives (group 0:
            # earliest start; group 7: shortest post-stream latency to M).
            g0parts, g7parts = [], []
            for pi in range(3):
                gp = gpool.tile([128, 1024], FP16, tag=f"g0p{pi}")
                src = bass.AP(tensor=sentd[:].tensor, offset=pi * 1024,
                              ap=[[NG * GW, 128], [1, 1024]])
                nc.sync.dma_start(gp, src)
                g0parts.append(gp)
            gtiles = [None]
            for g in range(1, NG - 1):
                gt = gpool.tile([128, GW], FP16, tag="gt")
                src = bass.AP(tensor=sentd[:].tensor, offset=g * GW,
                              ap=[[NG * GW, 128], [1, GW]])
                eng = nc.sync if g < 4 else nc.scalar
                eng.dma_start(gt, src)
                gtiles.append(gt)
            for pi in range(3):
                gp = gpool.tile([128, 1024], FP16, tag=f"g7p{pi}")
                src = bass.AP(tensor=sentd[:].tensor,
                              offset=(NG - 1) * GW + pi * 1024,
                              ap=[[NG * GW, 128], [1, 1024]])
                nc.scalar.dma_start(gp, src)
                g7parts.append(gp)
            gtiles.append(None)



            # ---------- persistent state (all 4x4 mats stored flat [16]) ----
            # TS[g][uu] = T_u^T: flat [j*4+k] = trans[k,j]+b[j]+e_u[j]
            TS = singles.tile([128, NG, UG, 16], F32)
            # Fn[g] = group total T_{g,0}o..oT_{g,3} natural [i*4+j]
            Fn = singles.tile([128, NG, 16], F32)
            # GpreN[g] = excl. prefix-of-groups natural [i*4+j]; [NG] = M
            GpreN = singles.tile([128, NG + 1, 16], F32)
            # GsufTc[g] = excl. suffix-of-groups transposed [x*4+i]
            GsufTc = singles.tile([128, NG, 16], F32)
            ab = singles.tile([128, 2, NG, 4], F32)   # [0]=asb, [1]=bsb
            alpha = singles.tile([128, NG, UG, 4], F32)
            beta = singles.tile([128, NG, UG, 4], F32)
            # p2 scan state: [0, s] = fwd alpha s_0..s_16; [1, c] = bwd bb_c
            states = singles.tile([BC, 2, C + 1, 4], F32)
            # ab2[0][c] = M_c natural [i*4+j]; ab2[1][c] = M_c^T [j*4+i]
            ab2 = singles.tile([BC, 2, C, 16], F32)
            sbc = singles.tile([128, 8], F32)   # cols 0:4 = sb_c, 4:8 = bb_c

            # prefills (off the Vector engine)
            nc.gpsimd.tensor_copy(GpreN[:, 0, :], mpid)
            nc.gpsimd.tensor_copy(GsufTc[:, NG - 1, :], mpid)
            nc.gpsimd.memset(states[:, 0, 0, :], 0.0)
            nc.scalar.copy(states[:, 1, C - 1, :], end_sb[0:BC, :])

            def compose(out_ab, A_am, BT_bm, pool, tag, nb=()):
                """out[.., a, b] = max_m A[.., a, m] + B[.., m, b]
                with BT = B^T ([.., b, m]). Both operands unit-stride in m."""
                shp = (128, *nb, 4, 4, 4)
                nd = len(shp)
                t4 = pool.tile(list(shp), F32, tag=tag)
                nc.vector.tensor_tensor(
                    t4,
                    A_am.unsqueeze(nd - 2).to_broadcast(shp),
                    BT_bm.unsqueeze(nd - 3).to_broadcast(shp),
                    OP.add)
                nc.vector.reduce_max(out_ab, t4, axis=AX.X)

            def m44(apv):
                return apv.rearrange("p (a m) -> p a m", a=4)

            # ---------- Stage A (software-pipelined) ----------
            # PE queue: mm(g) | tr(g-2) | mm(g+1) | tr(g-1) | ... -- the
            # transposes lag 2 groups so their eT_sb Scalar copy is long done
            # and the PE never stalls (stalls reset its p-state ramp).
            sA = nc.named_scope("stageA")
            sA.__enter__()
            eT_sbs, fx_tiles = {}, {}

            def emit_mm(g):
                eT_ps = ps_eT.tile([32, UG * 128], F32, tag="eT")
                for ch in range(HCH):
                    if g == 0:
                        rhs = g0parts[ch // 2][:, (ch % 2) * 512:(ch % 2) * 512 + 512]
                    elif g == NG - 1:
                        rhs = g7parts[ch // 2][:, (ch % 2) * 512:(ch % 2) * 512 + 512]
                    else:
                        rhs = gtiles[g].rearrange(
                            "p (ch n) -> p ch n", ch=HCH)[:, ch, :]
                    nc.tensor.matmul(eT_ps, wt[:, ch * 32:(ch + 1) * 32],
                                     rhs, start=(ch == 0),
                                     stop=(ch == HCH - 1))
                eT_sb = et_pool.tile([32, UG * 128], F32, tag="eTsb")
                nc.scalar.copy(eT_sb, eT_ps)
                eT_sbs[g] = eT_sb

            def emit_tr(g):
                fx_ps = ps_fx.tile([128, UG * 32], F32, tag="fx")
                eT_sb = eT_sbs.pop(g)
                for uu in range(UG):
                    nc.tensor.transpose(fx_ps[:, uu * 32:(uu + 1) * 32],
                                        eT_sb[:, uu * 128:(uu + 1) * 128], id32)
                fx_sb = et_pool.tile([128, UG * 32], F32, tag="fxsb")
                nc.scalar.copy(fx_sb, fx_ps)
                fx_tiles[g] = fx_sb

            def emit_chain(g):
                fxv = fx_tiles.pop(g).rearrange("p (u c) -> p u c", u=UG)
                # TS[g] = e_hi + e_lo + (trans+b)  (hi+lo on GpSimd: plain APs)
                th = gtmp_pool.tile([128, UG, 16], F32, tag="th")
                nc.gpsimd.tensor_tensor(th, fxv[:, :, 0:16],
                                        fxv[:, :, 16:32], OP.add)
                nc.vector.tensor_tensor(
                    TS[:, g, :, :], th,
                    tbrow.unsqueeze(1).to_broadcast((128, UG, 16)), OP.add)
                if g == 0:
                    # chunk-0 start fix (no-op +0.0 on rows c!=0)
                    nc.vector.tensor_tensor(TS[:, 0, 0, :], TS[:, 0, 0, :],
                                            rcfix, OP.add)
                tsg = TS[:, g, :, :]
                # ---- group total F_g = T0 o T1 o T2 o T3 (pair tree)
                # T_u natural [i][m] = TS[u][m*4+i] (transposed view)
                tnv = tsg.rearrange("p u (m i) -> p u m i", m=4) \
                         .transpose([0, 1, 3, 2])
                ff = gtmp_pool.tile([128, 2, 16], F32, tag="ff")
                compose(m44(ff[:, 0, :]), tnv[:, 0, :, :], m44(tsg[:, 1, :]),
                        gtmp_pool, "ff0")
                compose(m44(ff[:, 1, :]), tnv[:, 2, :, :], m44(tsg[:, 3, :]),
                        gtmp_pool, "ff1")
                # F = FF0 o FF1: BT = FF1^T (transposed view)
                compose(m44(Fn[:, g, :]), m44(ff[:, 0, :]),
                        m44(ff[:, 1, :]).transpose([0, 2, 1]),
                        gtmp_pool, "fg")
                # ---- running cross-group prefix: GpreN[g+1] = GpreN[g] o F_g
                compose(m44(GpreN[:, g + 1, :]), m44(GpreN[:, g, :]),
                        m44(Fn[:, g, :]).transpose([0, 2, 1]),
                        tmp_pool, "gpre")

            for g in range(NG):
                emit_mm(g)
                if g >= 1:
                    emit_tr(g - 1)
                    emit_chain(g - 1)
            emit_tr(NG - 1)
            emit_chain(NG - 1)
            sA.__exit__(None, None, None)

            # ---------- Gsuf: excl. suffix-of-groups (tail, serial) ----------
            # Split around p2: the first composes fill the ab2-DMA round trip,
            # the rest fill the sbc-DMA round trip (bsb is their only reader).
            _sg = nc.named_scope("gsuf")
            _sg.__enter__()
            for g in range(NG - 2, 2, -1):
                # GsufTc[g] = GsufTc[g+1] o F^T_{g+1}; (F^T)^T = F = Fn[g+1]
                compose(m44(GsufTc[:, g, :]), m44(GsufTc[:, g + 1, :]),
                        m44(Fn[:, g + 1, :]), tmp_pool, "gsuf")
            _sg.__exit__(None, None, None)

            # ---------- p2: per-sequence boundary scans over chunks ----------
            _sp2 = nc.named_scope("p2")
            _sp2.__enter__()
            # M_c natural by (b, c) rows -> ab2[1] (bwd form)
            nc.sync.dma_start(
                ab2[:, 1, :, :].rearrange("p c x -> p (c x)"), GpreN[:, NG, :])
            # transposed form (fwd) -> ab2[0] (Scalar, overlaps Gsuf)
            nc.scalar.copy(
                ab2[:, 0, :, :].rearrange("p c (j i) -> p c j i", j=4),
                ab2[:, 1, :, :].rearrange("p c (i j) -> p c i j", i=4)
                    .transpose([0, 1, 3, 2]))
            a2f = ab2[:, 0, :, :].rearrange("p c (j i) -> p c j i", j=4)
            a2b = ab2[:, 1, :, :].rearrange("p c (i j) -> p c i j", i=4)
            stv = states
            st_pd = states[:].ap[0]
            ab_pd = ab2[:].ap[0]
            st_t, ab_t = states[:].tensor, ab2[:].tensor
            def p2_std_fwd(s):
                tf = gtmp_pool.tile([BC, 4, 4], F32, tag="p2f")
                nc.vector.tensor_tensor(
                    tf,
                    stv[:, 0, s, :].unsqueeze(1).to_broadcast((BC, 4, 4)),
                    a2f[:, s, :, :], OP.add)
                nc.vector.reduce_max(stv[:, 0, s + 1, :], tf, axis=AX.X)

            def p2_std_bwd(s):
                cc = C - 1 - s
                tb = gtmp_pool.tile([BC, 4, 4], F32, tag="p2b")
                nc.vector.tensor_tensor(
                    tb,
                    stv[:, 1, cc, :].unsqueeze(1).to_broadcast((BC, 4, 4)),
                    a2b[:, cc, :, :], OP.add)
                nc.vector.reduce_max(stv[:, 1, cc - 1, :], tb, axis=AX.X)

            p2_std_fwd(0)
            p2_std_bwd(0)
            for s in range(1, C - 6):
                # fwd (d=0) + bwd (d=1) fused via pair APs with per-step
                # d-strides (all positive: fwd slots ascend, bwd descend)
                t2 = gtmp_pool.tile([BC, 2, 4, 4], F32, tag="p2x")
                in0 = bass.AP(tensor=st_t, offset=s * 4,
                              ap=[st_pd, [128 - 8 * s, 2], [0, 4], [1, 4]])
                in1 = bass.AP(tensor=ab_t, offset=s * 16,
                              ap=[ab_pd, [496 - 32 * s, 2], [4, 4], [1, 4]])
                nc.vector.tensor_tensor(t2, in0, in1, OP.add)
                out = bass.AP(tensor=st_t, offset=(s + 1) * 4,
                              ap=[st_pd, [120 - 8 * s, 2], [1, 4]])
                nc.vector.reduce_max(out, t2, axis=AX.X)
            # finish fwd first and fire its broadcast DMA while bwd wraps up,
            # so the DMA round trip overlaps the remaining bwd steps
            for s in range(C - 6, C - 1):
                p2_std_fwd(s)
            nc.sync.dma_start(
                sbc[:, 0:4], states[:, 0, 0:C, :].rearrange("p c x -> p (c x)"))
            for s in range(C - 6, C - 1):
                p2_std_bwd(s)
            nc.sync.dma_start(
                sbc[:, 4:8], states[:, 1, 0:C, :].rearrange("p c x -> p (c x)"))
            # remaining Gsuf composes fill the sbc round-trip latency
            for g in range(2, -1, -1):
                compose(m44(GsufTc[:, g, :]), m44(GsufTc[:, g + 1, :]),
                        m44(Fn[:, g + 1, :]), tmp_pool, "gsuf2")
            _sp2.__exit__(None, None, None)

            # ---------- boundary -> group -> step backfill ----------
            _sp3 = nc.named_scope("p3")
            _sp3.__enter__()
            # asb[g][j] = max_i sb[i] + Gpre[g][i][j]
            t4 = tmp_pool.tile([128, NG, 4, 4], F32, tag="asb")
            nc.vector.tensor_tensor(
                t4,
                GpreN[:, 0:NG, :].rearrange("p g (i j) -> p g i j", i=4)
                    .transpose([0, 1, 3, 2]),
                sbc[:, 0:4].unsqueeze(1).unsqueeze(1).to_broadcast((128, NG, 4, 4)),
                OP.add)
            nc.vector.reduce_max(ab[:, 0, :, :], t4, axis=AX.X)
            # alpha scan first (needs only the fwd boundary):
            #   alpha[(g,uu)][j] = max_i prev[i] + T_{g,uu}[i][j]
            for uu in range(UG):
                prev = (ab[:, 0, :, :] if uu == 0 else alpha[:, :, uu - 1, :])
                t4 = tmp_pool.tile([128, NG, 4, 4], F32, tag="al")
                nc.vector.tensor_tensor(
                    t4,
                    TS[:, :, uu, :].rearrange("p g (j i) -> p g j i", j=4),
                    prev.unsqueeze(2).to_broadcast((128, NG, 4, 4)),
                    OP.add)
                nc.vector.reduce_max(alpha[:, :, uu, :], t4, axis=AX.X)
            # bsb[g][i] = max_x Gsuf[g][i][x] + bb[x];  Gsuf[i][x] = GsufTc[x][i]
            t4 = tmp_pool.tile([128, NG, 4, 4], F32, tag="bsb")
            nc.vector.tensor_tensor(
                t4,
                GsufTc.rearrange("p g (x i) -> p g x i", x=4)
                    .transpose([0, 1, 3, 2]),
                sbc[:, 4:8].unsqueeze(1).unsqueeze(1).to_broadcast((128, NG, 4, 4)),
                OP.add)
            nc.vector.reduce_max(ab[:, 1, :, :], t4, axis=AX.X)
            # beta[(g,3)] = bsb[g]
            nc.scalar.copy(beta[:, :, 3, :], ab[:, 1, :, :])
            # beta scan: beta[(g,uu)][i] = max_x T_{g,uu+1}[i][x] + next[x]
            for uu in range(UG - 2, -1, -1):
                t4 = tmp_pool.tile([128, NG, 4, 4], F32, tag="be")
                nc.vector.tensor_tensor(
                    t4,
                    TS[:, :, uu + 1, :].rearrange("p g (x i) -> p g x i", x=4)
                        .transpose([0, 1, 3, 2]),
                    beta[:, :, uu + 1, :].unsqueeze(2)
                        .to_broadcast((128, NG, 4, 4)),
                    OP.add)
                nc.vector.reduce_max(beta[:, :, uu, :], t4, axis=AX.X)
            _sp3.__exit__(None, None, None)

            # ---------- tags: first-argmax_j(alpha+beta) ----------
            _sp5 = nc.named_scope("p5")
            _sp5.__enter__()
            delta = singles.tile([128, L, 4], F32)
            av = alpha.rearrange("p g u j -> p (g u) j")
            bv = beta.rearrange("p g u j -> p (g u) j")
            nc.vector.tensor_tensor(delta, av, bv, OP.add)
            # tie-bias -j*2^-17 makes is_equal pick the first max (matching
            # jnp.argmax) without the two-stage eq chain
            nc.vector.tensor_tensor(
                delta, delta, tiebias.unsqueeze(1).to_broadcast((128, L, 4)),
                OP.add)
            mx = tmp_pool.tile([128, L], F32, tag="mx")
            nc.vector.reduce_max(mx, delta, axis=AX.X)
            eq = singles.tile([128, L, 4], F32)
            nc.vector.tensor_tensor(
                eq, delta, mx.unsqueeze(2).to_broadcast((128, L, 4)), OP.is_equal)
            nc.vector.tensor_tensor(
                eq, eq, iw4.unsqueeze(1).to_broadcast((128, L, 4)), OP.mult)
            tagi = tmp_pool.tile([128, L], I32, tag="tagi")
            with nc.allow_low_precision(reason="0/1*j sum to int32 is exact"):
                nc.vector.reduce_sum(tagi, eq, axis=AX.X)
            nc.scalar.dma_start(tagsd[:].rearrange("b (c t) -> b c t", c=C), tagi)
            _sp5.__exit__(None, None, None)

    return nc


def _split_multi_waits(nc, waits_per_drain=1):
    """Walrus (bass2jax path) allows very few embedded sync waits per
    instruction (PE matmul: exactly 1). Hoist multi-waits onto standalone
    InstDrain instructions on the same engine, preserving order."""
    for f in nc.m.functions:
        for blk in f.blocks:
            insts = blk.instructions
            i = 0
            while i < len(insts):
                ins = insts[i]
                si = ins.sync_info
                w = list(si.on_wait) if (si is not None and si.on_wait) else []
                if len(w) >= 2:
                    groups = [w[j:j + waits_per_drain]
                              for j in range(0, len(w), waits_per_drain)]
                    for k, grp in enumerate(groups):
                        d = mybir.InstEventSemaphore(
                            name=nc.get_next_instruction_name(), ins=[], outs=[])
                        d.engine = ins.engine
                        d.sync_info = mybir.SyncInfo(on_wait=grp, on_update=[])
                        insts.insert(i + k, d)
                    i += len(groups)
                    ins.sync_info = mybir.SyncInfo(
                        on_wait=[], on_update=list(si.on_update or []))
                i += 1


def _get_nc():
    if "nc" not in _NC_CACHE:
        nc = build_nc()
        _split_multi_waits(nc, waits_per_drain=1)   # HW path only
        _NC_CACHE["nc"] = nc
    return _NC_CACHE["nc"]


def make_in_maps(inputs):
    sent = np.ascontiguousarray(np.asarray(inputs["sentences"], dtype=np.float32))
    W = np.ascontiguousarray(np.asarray(inputs["W"], dtype=np.float32))
    bb = np.asarray(inputs["b"], dtype=np.float32)
    st = np.asarray(inputs["start_transitions"], dtype=np.float32)
    en = np.asarray(inputs["end_transitions"], dtype=np.float32)
    tr = np.asarray(inputs["transitions"], dtype=np.float32)

    # W' lhsT: per h-chunk 32 cols: 0:16 = Whi[j] repl. over k, 16:32 = Wlo
    wT = np.transpose(W.reshape(K, HCH, 128), (2, 1, 0))   # [p, ch, j]
    whi = wT.astype(np.float16).astype(np.float32)
    wlo = (wT - whi).astype(np.float16).astype(np.float32)
    wt = np.zeros((128, HCH, 32), dtype=np.float32)
    wt[:, :, 0:16] = np.repeat(whi, 4, axis=2)
    wt[:, :, 16:32] = np.repeat(wlo, 4, axis=2)
    wt = np.ascontiguousarray(wt.reshape(128, WCOLS)).astype(np.float16)

    tbT = (tr.T + bb[:, None]).reshape(16).astype(np.float32)
    mpid = (np.where(np.eye(4, dtype=bool), 0.0, NEG)).astype(np.float32).ravel()
    fix = (st[:, None] - tr.T).reshape(16).astype(np.float32)

    rcm = np.zeros((128, 128), dtype=np.float32)
    rcm[:, 0:4] = [4.0, 3.0, 2.0, 1.0]
    rcm[:, 4:8] = [0.0, 1.0, 2.0, 3.0]
    rcm[:, 8:24] = mpid[None, :]
    rcm[:, 24:28] = en[None, :]
    rcm[0::C, 28:44] = fix[None, :]
    rcm[0:32, 44:76] = np.eye(32, dtype=np.float32)
    rcm[:, 92:108] = tbT[None, :]
    rcm[:, 108:112] = [0.0, -(2.0 ** -17), -2.0 * 2 ** -17, -3.0 * 2 ** -17]

    in_maps = []
    for core in range(NCORES):
        sc = sent[core * BC:(core + 1) * BC]           # [8, 512, 768]
        s6 = sc.reshape(BC, C, NG, UG, HCH, 128)       # b c g uu ch p
        sT = np.transpose(s6, (5, 2, 4, 3, 0, 1))      # p g ch uu b c
        sT = np.ascontiguousarray(sT.reshape(128, NG * GW)).astype(np.float16)
        in_maps.append({
            "sentT": sT, "wt": wt, "rowconsts": rcm,
        })
    return in_maps


def kernel(**inputs):
    nc = _get_nc()
    in_maps = make_in_maps(inputs)
    res = run_bass_kernel_spmd(nc, in_maps, core_ids=list(range(NCORES)))
    tags = np.concatenate([res.results[c]["tags"] for c in range(NCORES)], axis=0)
    return tags.astype(np.int32)


if __name__ == "__main__":
    import reference
    inputs = {k: np.asarray(v) for k, v in reference.setup_inputs().items()}
    out = kernel(**inputs)
    print(out.shape, out.dtype, out[:2, :16])


# revision 47
# speedup vs baseline: 1.0617x; 1.0617x over previous
"""BERT-CRF Viterbi decode kernel for Trainium2 (Bass/Tile), 8-core data parallel.

v3: transitions folded into the PE matmul; batched group-level max-plus
algebra with unit-stride compose ops throughout.

Full inputs in, full outputs out. Batch B=64 sharded across 8 cores (8 seqs
each). Per core, 128 partition rows = (b=8 seqs) x (c=16 chunks of L=32 steps);
each chunk splits into NG=8 groups of UG=4 steps.

  Host pre-transposes sentences to fp16 [p=h%128, (g, ch, uu, row)] and packs
  W as fp16 hi+lo (32 cols per h-chunk), so one PE pass per group produces
  eT[(j,k)hi|lo, (uu,row)] with trans[k,j]+b[j] riding a tiny hi/lo seed
  matmul against a DMA-loaded ones-row. PE transposes ([32,128] blocks)
  deliver per-row layout; one DVE add (hi+lo) finishes the fp32-grade step
  matrices TS[u][j][k] = T_u^T.

  All max-plus composes C = A o B are emitted as t[a,b,m] = A[a,m] + B^T[b,m]
  with unit innermost stride on BOTH operands and reduce_max over m -- the
  fast DVE pattern. Chains keep the recurrent operand on the A side; the
  static operand is pre-transposed (TS is already T^T; TN/PTs/STc2 come from
  cheap Scalar strided copies that hide under the DMA stream).

  Per group (hidden under the stream): P chain (3), S^T chain (2), running
  cross-group GpreN (1) = 6 composes. Tail: GsufTc (7 composes), boundary
  scans p2f/p2b over chunks, batched asb/bsb -> alpha/beta backfill ->
  tags = first-argmax_j(alpha_u[j] + beta_u[j]).
"""
import sys
for p in ("/opt/trn_rl_repo", "/root/.axon_site/_ro/trn_rl_repo"):
    if p not in sys.path:
        sys.path.append(p)

import numpy as np
import concourse.bass as bass
import concourse.tile as tile
from concourse import mybir
from concourse.bass_utils import run_bass_kernel_spmd

F32 = mybir.dt.float32
FP16 = mybir.dt.float16
I32 = mybir.dt.int32
AX = mybir.AxisListType
OP = mybir.AluOpType

B, T, H, K = 64, 512, 768, 4
NCORES = 8
BC = B // NCORES          # 8 sequences per core
C, L = 16, 32             # chunks per sequence, steps per chunk
ROWS = BC * C             # 128 partition rows
HCH = H // 128            # 6 h-chunks
UG = 4                    # steps per group
NG = L // UG              # 8 groups
GW = HCH * UG * 128       # 3072 fp16 cols per partition per group
WCOLS = HCH * 32          # W' lhsT cols: 6 chunks x (16 hi | 16 lo)

NEG = -1.0e30

_NC_CACHE = {}


def build_nc():
    nc = bass.Bass()
    sentd = nc.declare_dram_parameter("sentT", [128, NG * GW], FP16, isOutput=False)
    wtd = nc.declare_dram_parameter("wt", [128, WCOLS], FP16, isOutput=False)
    # rowconsts[128, 128]: wfirst | iw4 | mpid | end | rcfix | id32 | tbrow
    rcd = nc.declare_dram_parameter("rowconsts", [128, 128], F32, isOutput=False)
    tagsd = nc.declare_dram_parameter("tags", [BC, T], I32, isOutput=True)

    with tile.TileContext(nc) as tc:
        with tc.tile_pool(name="singles", bufs=1) as singles, \
             tc.tile_pool(name="gpool", bufs=NG) as gpool, \
             tc.tile_pool(name="et_pool", bufs=3) as et_pool, \
             tc.tile_pool(name="tmp_pool", bufs=2) as tmp_pool, \
             tc.tile_pool(name="gtmp_pool", bufs=2) as gtmp_pool, \
             tc.tile_pool(name="ps_eT", bufs=3, space="PSUM") as ps_eT, \
             tc.tile_pool(name="ps_fx", bufs=2, space="PSUM") as ps_fx:

            # ---------- constants (Act ring; Sync ring leads with group 0) ----
            wt = singles.tile([128, WCOLS], FP16)
            nc.scalar.dma_start(wt, wtd[:])
            rc = singles.tile([128, 128], F32)
            nc.scalar.dma_start(rc, rcd[:])
            wfirst = rc[:, 0:4]
            iw4 = rc[:, 4:8]
            mpid = rc[:, 8:24]          # max-plus identity (0 diag, NEG off)
            end_sb = rc[:, 24:28]
            rcfix = rc[:, 28:44]        # rows c==0: start[j]-trans[k,j]; else 0
            id32 = rc[0:32, 44:76]
            tbrow = rc[:, 92:108]       # tbrow[j*4+k] = trans[k,j] + b[j]
            tiebias = rc[:, 108:112]    # [-j * 2^-17 for j in 0..3]

            # ---------- prefetch all sentence groups ----------
            # Group 0 lands in 3 separate ch-pair tiles so the first matmuls
            # start as soon as their h-chunks arrive (per-tile dep tracking).
            g0parts = []
            for pi in range(3):
                gp = gpool.tile([128, 1024], FP16, tag=f"g0p{pi}")
                src = bass.AP(tensor=sentd[:].tensor, offset=pi * 1024,
                              ap=[[NG * GW, 128], [1, 1024]])
                nc.sync.dma_start(gp, src)
                g0parts.append(gp)
            gtiles = [None]
            for g in range(1, NG):
                gt = gpool.tile([128, GW], FP16, tag="gt")
                src = bass.AP(tensor=sentd[:].tensor, offset=g * GW,
                              ap=[[NG * GW, 128], [1, GW]])
                eng = nc.sync if g < 4 else nc.scalar
                eng.dma_start(gt, src)
                gtiles.append(gt)



            # ---------- persistent state (all 4x4 mats stored flat [16]) ----
            # TS[g][uu] = T_u^T: flat [j*4+k] = trans[k,j]+b[j]+e_u[j]
            TS = singles.tile([128, NG, UG, 16], F32)
            # Fn[g] = group total T_{g,0}o..oT_{g,3} natural [i*4+j]
            Fn = singles.tile([128, NG, 16], F32)
            # GpreN[g] = excl. prefix-of-groups natural [i*4+j]; [NG] = M
            GpreN = singles.tile([128, NG + 1, 16], F32)
            # GsufTc[g] = excl. suffix-of-groups transposed [x*4+i]
            GsufTc = singles.tile([128, NG, 16], F32)
            ab = singles.tile([128, 2, NG, 4], F32)   # [0]=asb, [1]=bsb
            alpha = singles.tile([128, NG, UG, 4], F32)
            beta = singles.tile([128, NG, UG, 4], F32)
            # p2 scan state: [0, s] = fwd alpha s_0..s_16; [1, c] = bwd bb_c
            states = singles.tile([BC, 2, C + 1, 4], F32)
            # ab2[0][c] = M_c natural [i*4+j]; ab2[1][c] = M_c^T [j*4+i]
            ab2 = singles.tile([BC, 2, C, 16], F32)
            sbc = singles.tile([128, 8], F32)   # cols 0:4 = sb_c, 4:8 = bb_c

            # prefills (off the Vector engine)
            nc.gpsimd.tensor_copy(GpreN[:, 0, :], mpid)
            nc.gpsimd.tensor_copy(GsufTc[:, NG - 1, :], mpid)
            nc.gpsimd.memset(states[:, 0, 0, :], 0.0)
            nc.scalar.copy(states[:, 1, C - 1, :], end_sb[0:BC, :])

            def compose(out_ab, A_am, BT_bm, pool, tag, nb=()):
                """out[.., a, b] = max_m A[.., a, m] + B[.., m, b]
                with BT = B^T ([.., b, m]). Both operands unit-stride in m."""
                shp = (128, *nb, 4, 4, 4)
                nd = len(shp)
                t4 = pool.tile(list(shp), F32, tag=tag)
                nc.vector.tensor_tensor(
                    t4,
                    A_am.unsqueeze(nd - 2).to_broadcast(shp),
                    BT_bm.unsqueeze(nd - 3).to_broadcast(shp),
                    OP.add)
                nc.vector.reduce_max(out_ab, t4, axis=AX.X)

            def m44(apv):
                return apv.rearrange("p (a m) -> p a m", a=4)

            # ---------- Stage A (software-pipelined) ----------
            # PE queue: mm(g) | tr(g-2) | mm(g+1) | tr(g-1) | ... -- the
            # transposes lag 2 groups so their eT_sb Scalar copy is long done
            # and the PE never stalls (stalls reset its p-state ramp).
            sA = nc.named_scope("stageA")
            sA.__enter__()
            eT_sbs, fx_tiles = {}, {}

            def emit_mm(g):
                eT_ps = ps_eT.tile([32, UG * 128], F32, tag="eT")
                for ch in range(HCH):
                    if g == 0:
                        rhs = g0parts[ch // 2][:, (ch % 2) * 512:(ch % 2) * 512 + 512]
                    else:
                        rhs = gtiles[g].rearrange(
                            "p (ch n) -> p ch n", ch=HCH)[:, ch, :]
                    nc.tensor.matmul(eT_ps, wt[:, ch * 32:(ch + 1) * 32],
                                     rhs, start=(ch == 0),
                                     stop=(ch == HCH - 1))
                eT_sb = et_pool.tile([32, UG * 128], F32, tag="eTsb")
                nc.scalar.copy(eT_sb, eT_ps)
                eT_sbs[g] = eT_sb

            def emit_tr(g):
                fx_ps = ps_fx.tile([128, UG * 32], F32, tag="fx")
                eT_sb = eT_sbs.pop(g)
                for uu in range(UG):
                    nc.tensor.transpose(fx_ps[:, uu * 32:(uu + 1) * 32],
                                        eT_sb[:, uu * 128:(uu + 1) * 128], id32)
                fx_sb = et_pool.tile([128, UG * 32], F32, tag="fxsb")
                nc.scalar.copy(fx_sb, fx_ps)
                fx_tiles[g] = fx_sb

            def emit_chain(g):
                fxv = fx_tiles.pop(g).rearrange("p (u c) -> p u c", u=UG)
                # TS[g] = e_hi + e_lo + (trans+b)  (hi+lo on GpSimd: plain APs)
                th = gtmp_pool.tile([128, UG, 16], F32, tag="th")
                nc.gpsimd.tensor_tensor(th, fxv[:, :, 0:16],
                                        fxv[:, :, 16:32], OP.add)
                nc.vector.tensor_tensor(
                    TS[:, g, :, :], th,
                    tbrow.unsqueeze(1).to_broadcast((128, UG, 16)), OP.add)
                if g == 0:
                    # chunk-0 start fix (no-op +0.0 on rows c!=0)
                    nc.vector.tensor_tensor(TS[:, 0, 0, :], TS[:, 0, 0, :],
                                            rcfix, OP.add)
                tsg = TS[:, g, :, :]
                # ---- group total F_g = T0 o T1 o T2 o T3 (pair tree)
                # T_u natural [i][m] = TS[u][m*4+i] (transposed view)
                tnv = tsg.rearrange("p u (m i) -> p u m i", m=4) \
                         .transpose([0, 1, 3, 2])
                ff = gtmp_pool.tile([128, 2, 16], F32, tag="ff")
                compose(m44(ff[:, 0, :]), tnv[:, 0, :, :], m44(tsg[:, 1, :]),
                        gtmp_pool, "ff0")
                compose(m44(ff[:, 1, :]), tnv[:, 2, :, :], m44(tsg[:, 3, :]),
                        gtmp_pool, "ff1")
                # F = FF0 o FF1: BT = FF1^T (transposed view)
                compose(m44(Fn[:, g, :]), m44(ff[:, 0, :]),
                        m44(ff[:, 1, :]).transpose([0, 2, 1]),
                        gtmp_pool, "fg")
                # ---- running cross-group prefix: GpreN[g+1] = GpreN[g] o F_g
                compose(m44(GpreN[:, g + 1, :]), m44(GpreN[:, g, :]),
                        m44(Fn[:, g, :]).transpose([0, 2, 1]),
                        tmp_pool, "gpre")

            for g in range(NG):
                emit_mm(g)
                if g >= 1:
                    emit_tr(g - 1)
                    emit_chain(g - 1)
            emit_tr(NG - 1)
            emit_chain(NG - 1)
            sA.__exit__(None, None, None)

            # ---------- Gsuf: excl. suffix-of-groups (tail, serial) ----------
            # Split around p2: the first composes fill the ab2-DMA round trip,
            # the rest fill the sbc-DMA round trip (bsb is their only reader).
            _sg = nc.named_scope("gsuf")
            _sg.__enter__()
            for g in range(NG - 2, 2, -1):
                # GsufTc[g] = GsufTc[g+1] o F^T_{g+1}; (F^T)^T = F = Fn[g+1]
                compose(m44(GsufTc[:, g, :]), m44(GsufTc[:, g + 1, :]),
                        m44(Fn[:, g + 1, :]), tmp_pool, "gsuf")
            _sg.__exit__(None, None, None)

            # ---------- p2: per-sequence boundary scans over chunks ----------
            _sp2 = nc.named_scope("p2")
            _sp2.__enter__()
            # M_c natural by (b, c) rows -> ab2[1] (bwd form)
            nc.sync.dma_start(
                ab2[:, 1, :, :].rearrange("p c x -> p (c x)"), GpreN[:, NG, :])
            # transposed form (fwd) -> ab2[0] (Scalar, overlaps Gsuf)
            nc.scalar.copy(
                ab2[:, 0, :, :].rearrange("p c (j i) -> p c j i", j=4),
                ab2[:, 1, :, :].rearrange("p c (i j) -> p c i j", i=4)
                    .transpose([0, 1, 3, 2]))
            a2f = ab2[:, 0, :, :].rearrange("p c (j i) -> p c j i", j=4)
            a2b = ab2[:, 1, :, :].rearrange("p c (i j) -> p c i j", i=4)
            stv = states
            st_pd = states[:].ap[0]
            ab_pd = ab2[:].ap[0]
            st_t, ab_t = states[:].tensor, ab2[:].tensor
            def p2_std_fwd(s):
                tf = gtmp_pool.tile([BC, 4, 4], F32, tag="p2f")
                nc.vector.tensor_tensor(
                    tf,
                    stv[:, 0, s, :].unsqueeze(1).to_broadcast((BC, 4, 4)),
                    a2f[:, s, :, :], OP.add)
                nc.vector.reduce_max(stv[:, 0, s + 1, :], tf, axis=AX.X)

            def p2_std_bwd(s):
                cc = C - 1 - s
                tb = gtmp_pool.tile([BC, 4, 4], F32, tag="p2b")
                nc.vector.tensor_tensor(
                    tb,
                    stv[:, 1, cc, :].unsqueeze(1).to_broadcast((BC, 4, 4)),
                    a2b[:, cc, :, :], OP.add)
                nc.vector.reduce_max(stv[:, 1, cc - 1, :], tb, axis=AX.X)

            p2_std_fwd(0)
            p2_std_bwd(0)
            for s in range(1, C - 3):
                # fwd (d=0) + bwd (d=1) fused via pair APs with per-step
                # d-strides (all positive: fwd slots ascend, bwd descend)
                t2 = gtmp_pool.tile([BC, 2, 4, 4], F32, tag="p2x")
                in0 = bass.AP(tensor=st_t, offset=s * 4,
                              ap=[st_pd, [128 - 8 * s, 2], [0, 4], [1, 4]])
                in1 = bass.AP(tensor=ab_t, offset=s * 16,
                              ap=[ab_pd, [496 - 32 * s, 2], [4, 4], [1, 4]])
                nc.vector.tensor_tensor(t2, in0, in1, OP.add)
                out = bass.AP(tensor=st_t, offset=(s + 1) * 4,
                              ap=[st_pd, [120 - 8 * s, 2], [1, 4]])
                nc.vector.reduce_max(out, t2, axis=AX.X)
            # finish fwd first and fire its broadcast DMA while bwd wraps up
            p2_std_fwd(C - 3)
            p2_std_fwd(C - 2)
            nc.sync.dma_start(
                sbc[:, 0:4], states[:, 0, 0:C, :].rearrange("p c x -> p (c x)"))
            p2_std_bwd(C - 3)
            p2_std_bwd(C - 2)
            nc.sync.dma_start(
                sbc[:, 4:8], states[:, 1, 0:C, :].rearrange("p c x -> p (c x)"))
            # remaining Gsuf composes fill the sbc round-trip latency
            for g in range(2, -1, -1):
                compose(m44(GsufTc[:, g, :]), m44(GsufTc[:, g + 1, :]),
                        m44(Fn[:, g + 1, :]), tmp_pool, "gsuf2")
            _sp2.__exit__(None, None, None)

            # ---------- boundary -> group -> step backfill ----------
            _sp3 = nc.named_scope("p3")
            _sp3.__enter__()
            # asb[g][j] = max_i sb[i] + Gpre[g][i][j]
            t4 = tmp_pool.tile([128, NG, 4, 4], F32, tag="asb")
            nc.vector.tensor_tensor(
                t4,
                GpreN[:, 0:NG, :].rearrange("p g (i j) -> p g i j", i=4)
                    .transpose([0, 1, 3, 2]),
                sbc[:, 0:4].unsqueeze(1).unsqueeze(1).to_broadcast((128, NG, 4, 4)),
                OP.add)
            nc.vector.reduce_max(ab[:, 0, :, :], t4, axis=AX.X)
            # alpha scan first (needs only the fwd boundary):
            #   alpha[(g,uu)][j] = max_i prev[i] + T_{g,uu}[i][j]
            for uu in range(UG):
                prev = (ab[:, 0, :, :] if uu == 0 else alpha[:, :, uu - 1, :])
                t4 = tmp_pool.tile([128, NG, 4, 4], F32, tag="al")
                nc.vector.tensor_tensor(
                    t4,
                    TS[:, :, uu, :].rearrange("p g (j i) -> p g j i", j=4),
                    prev.unsqueeze(2).to_broadcast((128, NG, 4, 4)),
                    OP.add)
                nc.vector.reduce_max(alpha[:, :, uu, :], t4, axis=AX.X)
            # bsb[g][i] = max_x Gsuf[g][i][x] + bb[x];  Gsuf[i][x] = GsufTc[x][i]
            t4 = tmp_pool.tile([128, NG, 4, 4], F32, tag="bsb")
            nc.vector.tensor_tensor(
                t4,
                GsufTc.rearrange("p g (x i) -> p g x i", x=4)
                    .transpose([0, 1, 3, 2]),
                sbc[:, 4:8].unsqueeze(1).unsqueeze(1).to_broadcast((128, NG, 4, 4)),
                OP.add)
            nc.vector.reduce_max(ab[:, 1, :, :], t4, axis=AX.X)
            # beta[(g,3)] = bsb[g]
            nc.scalar.copy(beta[:, :, 3, :], ab[:, 1, :, :])
            # beta scan: beta[(g,uu)][i] = max_x T_{g,uu+1}[i][x] + next[x]
            for uu in range(UG - 2, -1, -1):
                t4 = tmp_pool.tile([128, NG, 4, 4], F32, tag="be")
                nc.vector.tensor_tensor(
                    t4,
                    TS[:, :, uu + 1, :].rearrange("p g (x i) -> p g x i", x=4)
                        .transpose([0, 1, 3, 2]),
                    beta[:, :, uu + 1, :].unsqueeze(2)
                        .to_broadcast((128, NG, 4, 4)),
                    OP.add)
                nc.vector.reduce_max(beta[:, :, uu, :], t4, axis=AX.X)
            _sp3.__exit__(None, None, None)

            # ---------- tags: first-argmax_j(alpha+beta) ----------
            _sp5 = nc.named_scope("p5")
            _sp5.__enter__()
            delta = singles.tile([128, L, 4], F32)
            av = alpha.rearrange("p g u j -> p (g u) j")
            bv = beta.rearrange("p g u j -> p (g u) j")
            nc.vector.tensor_tensor(delta, av, bv, OP.add)
            # tie-bias -j*2^-17 makes is_equal pick the first max (matching
            # jnp.argmax) without the two-stage eq chain
            nc.vector.tensor_tensor(
                delta, delta, tiebias.unsqueeze(1).to_broadcast((128, L, 4)),
                OP.add)
            mx = tmp_pool.tile([128, L], F32, tag="mx")
            nc.vector.reduce_max(mx, delta, axis=AX.X)
            eq = singles.tile([128, L, 4], F32)
            nc.vector.tensor_tensor(
                eq, delta, mx.unsqueeze(2).to_broadcast((128, L, 4)), OP.is_equal)
            nc.vector.tensor_tensor(
                eq, eq, iw4.unsqueeze(1).to_broadcast((128, L, 4)), OP.mult)
            tagi = tmp_pool.tile([128, L], I32, tag="tagi")
            with nc.allow_low_precision(reason="0/1*j sum to int32 is exact"):
                nc.vector.reduce_sum(tagi, eq, axis=AX.X)
            nc.scalar.dma_start(tagsd[:].rearrange("b (c t) -> b c t", c=C), tagi)
            _sp5.__exit__(None, None, None)

    return nc


def _split_multi_waits(nc, waits_per_drain=1):
    """Walrus (bass2jax path) allows very few embedded sync waits per
    instruction (PE matmul: exactly 1). Hoist multi-waits onto standalone
    InstDrain instructions on the same engine, preserving order."""
    for f in nc.m.functions:
        for blk in f.blocks:
            insts = blk.instructions
            i = 0
            while i < len(insts):
                ins = insts[i]
                si = ins.sync_info
                w = list(si.on_wait) if (si is not None and si.on_wait) else []
                if len(w) >= 2:
                    groups = [w[j:j + waits_per_drain]
                              for j in range(0, len(w), waits_per_drain)]
                    for k, grp in enumerate(groups):
                        d = mybir.InstEventSemaphore(
                            name=nc.get_next_instruction_name(), ins=[], outs=[])
                        d.engine = ins.engine
                        d.sync_info = mybir.SyncInfo(on_wait=grp, on_update=[])
                        insts.insert(i + k, d)
                    i += len(groups)
                    ins.sync_info = mybir.SyncInfo(
                        on_wait=[], on_update=list(si.on_update or []))
                i += 1


def _get_nc():
    if "nc" not in _NC_CACHE:
        nc = build_nc()
        _split_multi_waits(nc, waits_per_drain=1)   # HW path only
        _NC_CACHE["nc"] = nc
    return _NC_CACHE["nc"]


def make_in_maps(inputs):
    sent = np.ascontiguousarray(np.asarray(inputs["sentences"], dtype=np.float32))
    W = np.ascontiguousarray(np.asarray(inputs["W"], dtype=np.float32))
    bb = np.asarray(inputs["b"], dtype=np.float32)
    st = np.asarray(inputs["start_transitions"], dtype=np.float32)
    en = np.asarray(inputs["end_transitions"], dtype=np.float32)
    tr = np.asarray(inputs["transitions"], dtype=np.float32)

    # W' lhsT: per h-chunk 32 cols: 0:16 = Whi[j] repl. over k, 16:32 = Wlo
    wT = np.transpose(W.reshape(K, HCH, 128), (2, 1, 0))   # [p, ch, j]
    whi = wT.astype(np.float16).astype(np.float32)
    wlo = (wT - whi).astype(np.float16).astype(np.float32)
    wt = np.zeros((128, HCH, 32), dtype=np.float32)
    wt[:, :, 0:16] = np.repeat(whi, 4, axis=2)
    wt[:, :, 16:32] = np.repeat(wlo, 4, axis=2)
    wt = np.ascontiguousarray(wt.reshape(128, WCOLS)).astype(np.float16)

    tbT = (tr.T + bb[:, None]).reshape(16).astype(np.float32)
    mpid = (np.where(np.eye(4, dtype=bool), 0.0, NEG)).astype(np.float32).ravel()
    fix = (st[:, None] - tr.T).reshape(16).astype(np.float32)

    rcm = np.zeros((128, 128), dtype=np.float32)
    rcm[:, 0:4] = [4.0, 3.0, 2.0, 1.0]
    rcm[:, 4:8] = [0.0, 1.0, 2.0, 3.0]
    rcm[:, 8:24] = mpid[None, :]
    rcm[:, 24:28] = en[None, :]
    rcm[0::C, 28:44] = fix[None, :]
    rcm[0:32, 44:76] = np.eye(32, dtype=np.float32)
    rcm[:, 92:108] = tbT[None, :]
    rcm[:, 108:112] = [0.0, -(2.0 ** -17), -2.0 * 2 ** -17, -3.0 * 2 ** -17]

    in_maps = []
    for core in range(NCORES):
        sc = sent[core * BC:(core + 1) * BC]           # [8, 512, 768]
        s6 = sc.reshape(BC, C, NG, UG, HCH, 128)       # b c g uu ch p
        sT = np.transpose(s6, (5, 2, 4, 3, 0, 1))      # p g ch uu b c
        sT = np.ascontiguousarray(sT.reshape(128, NG * GW)).astype(np.float16)
        in_maps.append({
            "sentT": sT, "wt": wt, "rowconsts": rcm,
        })
    return in_maps


def kernel(**inputs):
    nc = _get_nc()
    in_maps = make_in_maps(inputs)
    res = run_bass_kernel_spmd(nc, in_maps, core_ids=list(range(NCORES)))
    tags = np.concatenate([res.results[c]["tags"] for c in range(NCORES)], axis=0)
    return tags.astype(np.int32)


if __name__ == "__main__":
    import reference
    inputs = {k: np.asarray(v) for k, v in reference.setup_inputs().items()}
    out = kernel(**inputs)
    print(out.shape, out.dtype, out[:2, :16])
